# revision 9
# baseline (speedup 1.0000x reference)
"""MoE router gate kernel for Trainium2 (8 NeuronCores, SPMD data-parallel).

Reference computation (per problem nn_Gate_7241314861587):
    logits = x @ weight.T          # [8192, 4096] @ [4096, 256] -> [8192, 256]
    scores = sigmoid(logits)
    topv, indices = top_k(scores, 8)
    gates = topv / sum(topv)
    returns (gates f32 [8192, 8], indices int32 [8192, 8])

Strategy:
  - Data parallel: 1024 tokens per core; router weight replicated.
  - Host prepacks x and w into transposed (contraction-on-partition) fp16
    hi/lo splits.  logits = xh@wh + xh@wl + xl@wh accumulated in fp32 PSUM
    gives fp32-equivalent precision (~1e-6 abs err on logits; exact top-8
    indices) at fp16 matmul speed (3 cycles/row vs 4 for native fp32).
  - Weights stay SBUF-resident as [128, 32, 512] (wh ++ wl concat on the
    free axis) so the xh matmul covers both wh and wl halves in a single
    512-wide moving pass; xl@wh accumulates into the left half; one DVE
    add folds the halves.
  - Top-8 via the DVE MAX8 / FIND_INDEX_8 hardware (nc.vector.max /
    max_index): one instruction each per 128-token tile.
"""

import numpy as np

TOKENS, DIM, N_EXPERTS, TOPK = 8192, 4096, 256, 8
N_CORES = 8
TOK_SHARD = TOKENS // N_CORES     # 1024
TT = TOK_SHARD // 128             # 8 token tiles per core
KC = DIM // 128                   # 32 contraction chunks

_HALF = np.float16

_compiled = None


def _build():
    import concourse.mybir as mybir
    import concourse.tile as tile
    from concourse import bacc

    f32 = mybir.dt.float32
    f16 = mybir.dt.float16
    u32 = mybir.dt.uint32

    nc = bacc.Bacc("TRN2", target_bir_lowering=False, debug=False)

    xh_d = nc.dram_tensor("xh", [TT, 128, KC * 128], f16, kind="ExternalInput")
    xl_d = nc.dram_tensor("xl", [TT, 128, KC * 128], f16, kind="ExternalInput")
    w_d = nc.dram_tensor("wcat", [128, KC * 512], f16, kind="ExternalInput")
    gates_d = nc.dram_tensor("gates", [TOK_SHARD, TOPK], f32, kind="ExternalOutput")
    idx_d = nc.dram_tensor("idx", [TOK_SHARD, TOPK], u32, kind="ExternalOutput")

    with tile.TileContext(nc) as tc:
        with (
            tc.tile_pool(name="wp", bufs=1) as wp,
            tc.tile_pool(name="xp", bufs=4) as xp,
            tc.tile_pool(name="pp", bufs=4, space="PSUM") as pp,
            tc.tile_pool(name="sp", bufs=2) as sp,
        ):
            # Weight resident in SBUF; loaded in 8 chunks so the first
            # matmuls only wait on the first 512 KB, not the full 4 MB.
            wt = wp.tile([128, KC, 512], f16, tag="w")
            w_view = w_d[:].rearrange("p (kc e) -> p kc e", kc=KC)
            WCHUNK = 4
            for i, kc0 in enumerate(range(0, KC, WCHUNK)):
                eng = nc.sync if i % 2 == 0 else nc.scalar
                eng.dma_start(
                    wt[:, kc0:kc0 + WCHUNK, :], w_view[:, kc0:kc0 + WCHUNK, :]
                )

            for t in range(TT):
                xh_t = xp.tile([128, KC, 128], f16, tag="xh")
                xl_t = xp.tile([128, KC, 128], f16, tag="xl")
                XCHUNK = 8
                for kc0 in range(0, KC, XCHUNK):
                    nc.sync.dma_start(
                        xh_t[:, kc0:kc0 + XCHUNK, :],
                        xh_d[t].rearrange("p (kc n) -> p kc n", kc=KC)[
                            :, kc0:kc0 + XCHUNK, :
                        ],
                    )
                    nc.scalar.dma_start(
                        xl_t[:, kc0:kc0 + XCHUNK, :],
                        xl_d[t].rearrange("p (kc n) -> p kc n", kc=KC)[
                            :, kc0:kc0 + XCHUNK, :
                        ],
                    )

                # logits_hh ++ logits_hl accumulate in one 512-wide bank;
                # xl@wh folds into the left half.  One LDW per matmul, and
                # the xh pass covers both weight halves per instruction.
                ps = pp.tile([128, 512], f32, tag="ps")
                for k in range(KC):
                    if k > 0:
                        nc.tensor.matmul(
                            ps[:, 0:256], xl_t[:, k - 1, :], wt[:, k - 1, 0:256],
                            start=False, stop=False, skip_group_check=True,
                        )
                    nc.tensor.matmul(
                        ps[:], xh_t[:, k, :], wt[:, k, :],
                        start=(k == 0), stop=(k == KC - 1),
                        skip_group_check=True,
                    )
                nc.tensor.matmul(
                    ps[:, 0:256], xl_t[:, KC - 1, :], wt[:, KC - 1, 0:256],
                    start=False, stop=False, skip_group_check=True,
                )

                hl = sp.tile([128, 256], f32, tag="hl")
                nc.scalar.activation(
                    hl[:], ps[:, 256:512], mybir.ActivationFunctionType.Copy
                )
                pre = sp.tile([128, 256], f32, tag="pre")
                nc.vector.tensor_add(pre[:], ps[:, 0:256], hl[:])
                scores = sp.tile([128, 256], f32, tag="scores")
                nc.scalar.activation(
                    scores[:], pre[:], mybir.ActivationFunctionType.Sigmoid
                )

                top = sp.tile([128, TOPK], f32, tag="top")
                idxt = sp.tile([128, TOPK], u32, tag="idxt")
                nc.vector.max(out=top[:], in_=scores[:])
                nc.vector.max_index(out=idxt[:], in_max=top[:], in_values=scores[:])

                ssum = sp.tile([128, 1], f32, tag="ssum")
                nc.vector.reduce_sum(ssum[:], top[:], axis=mybir.AxisListType.X)
                rec = sp.tile([128, 1], f32, tag="rec")
                nc.vector.reciprocal(rec[:], ssum[:])
                gt = sp.tile([128, TOPK], f32, tag="gt")
                nc.vector.tensor_scalar_mul(gt[:], top[:], rec[:])

                nc.sync.dma_start(gates_d[t * 128:(t + 1) * 128, :], gt[:])
                nc.sync.dma_start(idx_d[t * 128:(t + 1) * 128, :], idxt[:])

    nc.compile()
    return nc


def _prep_inputs(x, weight):
    """Host-side shard + transpose + fp16 hi/lo split -> per-core in_maps."""
    x = np.ascontiguousarray(np.asarray(x, dtype=np.float32))
    w = np.ascontiguousarray(np.asarray(weight, dtype=np.float32))

    # Weight: wcat[p, kc*512 + e'] with e' = [wh(256) ++ wl(256)]
    wT = np.ascontiguousarray(w.T)                     # [4096, 256]
    wh = wT.astype(_HALF)
    wl = (wT - wh.astype(np.float32)).astype(_HALF)
    wcat = np.concatenate([wh, wl], axis=1)            # [4096, 512]
    wcat = wcat.reshape(KC, 128, 512).transpose(1, 0, 2).reshape(128, KC * 512)
    wcat = np.ascontiguousarray(wcat)

    xh = x.astype(_HALF)
    xl = (x - xh.astype(np.float32)).astype(_HALF)

    in_maps = []
    for c in range(N_CORES):
        sl = slice(c * TOK_SHARD, (c + 1) * TOK_SHARD)
        maps = {}
        for name, arr in (("xh", xh[sl]), ("xl", xl[sl])):
            # [1024, 4096] -> [t, tok, kc, p] -> [t, p, kc, tok]
            a = arr.reshape(TT, 128, KC, 128).transpose(0, 3, 2, 1)
            maps[name] = np.ascontiguousarray(a.reshape(TT, 128, KC * 128))
        maps["wcat"] = wcat
        in_maps.append(maps)
    return in_maps


def kernel(x, weight, _trace=False, _trace_kwargs=None):
    global _compiled
    from concourse.bass_utils import run_bass_kernel_spmd

    if _compiled is None:
        _compiled = _build()

    in_maps = _prep_inputs(x, weight)
    res = run_bass_kernel_spmd(
        _compiled,
        in_maps,
        core_ids=list(range(N_CORES)),
        trace=_trace,
        **(_trace_kwargs or {}),
    )

    gates = np.concatenate([r["gates"] for r in res.results], axis=0)
    idx = np.concatenate(
        [r["idx"].astype(np.int32) for r in res.results], axis=0
    )
    if _trace:
        kernel.last_results = res
    return gates, idx
